# revision 4
# baseline (speedup 1.0000x reference)
"""Device computes only the dense interior conv (99.2% of FLOPs) at full PE
rate; the 1-pixel border frame (4 edges + corners, 8 different weight sets)
is computed on host in f32. Flat pad-free strip layout keeps every matmul rhs
contiguous; bf16 output DMA; PE warmup burst; per-group output DMAs on the
last strip to shorten the drain tail.
"""

import ml_dtypes
import numpy as np

import concourse.bacc as bacc
import concourse.mybir as mybir
import concourse.tile as tile
from concourse.bass import ts
from concourse.bass_utils import run_bass_kernel_spmd

B, C, H, W = 8, 64, 256, 256
NCORES = 8
R = 32              # output rows per strip
H2 = R // 2         # rows per partition-half
NSTRIP = H // R
SLOTS = H2 + 2      # rows stored per half (1 halo row each side)
XCOLS = SLOTS * W + 2   # flat, no pad columns; +2 guard elements
NG = H2 // 4        # groups of 4 row-pairs per half
F32 = mybir.dt.float32
BF16 = mybir.dt.bfloat16
AF = mybir.ActivationFunctionType
BF = ml_dtypes.bfloat16

TAPS9 = [(dy, dx) for dy in (-1, 0, 1) for dx in (-1, 0, 1)]
NW = 9

_CACHE = {}


def _build():
    nc = bacc.Bacc("TRN2", target_bir_lowering=False, debug=False,
                   num_devices=NCORES)
    ip = nc.dram_tensor("img_prep", [NSTRIP, 128, XCOLS], BF16,
                        kind="ExternalInput").ap()
    wt_d = nc.dram_tensor("wt", [128, NW * 64], BF16, kind="ExternalInput").ap()
    bias_d = nc.dram_tensor("bias", [128, 1], F32, kind="ExternalInput").ap()
    out_d = nc.dram_tensor("out", [NSTRIP, 2, 128, H2 * W // 2], BF16,
                           kind="ExternalOutput").ap()

    lo, up = slice(0, 64), slice(64, 128)

    with tile.TileContext(nc) as tc:
        with (
            tc.tile_pool(name="const", bufs=1) as constp,
            tc.tile_pool(name="xin", bufs=3) as xp,
            tc.tile_pool(name="outp", bufs=3) as op,
            tc.tile_pool(name="psmain", bufs=4, space="PSUM") as pp,
        ):
            wt = constp.tile([128, NW * 64], BF16)
            nc.sync.dma_start(wt[:], wt_d[:])
            X0 = xp.tile([128, XCOLS], BF16)
            nc.sync.dma_start(X0[:], ip[0])
            bias_t = constp.tile([128, 1], F32)
            nc.sync.dma_start(bias_t[:], bias_d[:])
            wtr = wt[:]

            # ---- PE warmup while strip 0 streams in ----
            for wu in range(8):
                pw = pp.tile([128, 512], F32, tag="ps1")
                st0 = 64 * (wu % 2)
                nc.tensor.matmul(pw[:], wtr[:, 0:128],
                                 wtr[:, st0: st0 + 512],
                                 start=True, stop=True, skip_group_check=True)

            # ---- dense interior conv (d=4) ----
            for s in range(NSTRIP):
                if s == 0:
                    X = X0
                else:
                    X = xp.tile([128, XCOLS], BF16)
                    nc.sync.dma_start(X[:], ip[s])
                Xf = X[:]
                olo = op.tile([128, H2 * W // 2], BF16, tag="olo")
                oup = op.tile([128, H2 * W // 2], BF16, tag="oup")

                for g in range(NG):
                    ps1 = pp.tile([128, 512], F32, tag="ps1")
                    ps2 = pp.tile([128, 512], F32, tag="ps2")
                    for k, (dy, dx) in enumerate(TAPS9):
                        st, sp = (k == 0), (k == 8)
                        for (ph, po, u) in ((lo, slice(0, 64), 4 * g),
                                            (up, slice(0, 64), 4 * g),
                                            (lo, slice(64, 128), 4 * g + 2),
                                            (up, slice(64, 128), 4 * g + 2)):
                            psd = (ps1 if ph == lo else ps2)
                            base = (u + dy + 1) * W + dx + 1
                            rhs = Xf[ph, base: base + 512]
                            nc.tensor.matmul(
                                psd[po, :],
                                wtr[ph, ts(k, 64)], rhs,
                                start=st, stop=sp, skip_group_check=True)
                    # evacuate: bias add psum -> sbuf bf16; alternate engines
                    blo = bias_t[:, 0:1]
                    if g % 2 == 0:
                        nc.scalar.activation(olo[:, ts(g, 512)], ps1[:],
                                             AF.Identity, bias=blo)
                        nc.vector.tensor_scalar_add(oup[:, ts(g, 512)],
                                                    ps2[:], blo)
                    else:
                        nc.vector.tensor_scalar_add(olo[:, ts(g, 512)],
                                                    ps1[:], blo)
                        nc.scalar.activation(oup[:, ts(g, 512)], ps2[:],
                                             AF.Identity, bias=blo)

                if s == NSTRIP - 1:
                    nc.sync.dma_start(out_d[s, 0, :, 0:1024], olo[:, 0:1024])
                    nc.sync.dma_start(out_d[s, 1, :, 0:1024], oup[:, 0:1024])
                    nc.sync.dma_start(out_d[s, 0, :, 1024:2048],
                                      olo[:, 1024:2048])
                    nc.sync.dma_start(out_d[s, 1, :, 1024:2048],
                                      oup[:, 1024:2048])
                else:
                    nc.sync.dma_start(out_d[s, 0], olo[:])
                    nc.sync.dma_start(out_d[s, 1], oup[:])

    nc.compile()
    return nc


def _get_nc():
    if "nc" not in _CACHE:
        _CACHE["nc"] = _build()
    return _CACHE["nc"]


def _prep_img(imgc):
    """[64,256,256] f32 -> [NSTRIP,128,XCOLS] flat bf16 strip layout."""
    ipk = np.zeros((NSTRIP, 2, 64, XCOLS), BF)
    for s in range(NSTRIP):
        for g in range(2):
            base = s * R + g * H2 - 1    # first stored row (halo)
            l0 = max(0, -base)
            h0 = min(SLOTS, H - base)
            ipk[s, g, :, 1 + l0 * W: 1 + h0 * W] = \
                imgc[:, base + l0: base + h0, :].reshape(64, -1)
    return np.ascontiguousarray(ipk.reshape(NSTRIP, 128, XCOLS))


def _prep_wt(weights):
    wt = np.zeros((128, NW, 64), BF)
    for k, (dy, dx) in enumerate(TAPS9):
        m = weights[4][:, :, dy + 1, dx + 1].T  # [cin, cout]
        wt[0:64, k] = m
        wt[64:128, k] = m
    return np.ascontiguousarray(wt.reshape(128, NW * 64))


def _prep_bias(bias):
    bs = np.zeros((128, 1), np.float32)
    bs[0:64, 0] = bias[4]
    bs[64:128, 0] = bias[4]
    return bs


def _borders(imgc, weights, bias):
    """Host-side f32 border frame.

    Returns (top[64,254], bot[64,254], left[64,254], right[64,254],
    corners[4,64]) for the 1-pixel frame: top/bot cover x=1..254 at y=0/255,
    left/right cover y=1..254 at x=0/255.
    """
    w = weights
    top = np.zeros((64, 254), np.float32)
    bot = np.zeros((64, 254), np.float32)
    lef = np.zeros((64, 254), np.float32)
    rig = np.zeros((64, 254), np.float32)
    for dy in (0, 1):                      # d=7 (top row)
        for dx in (-1, 0, 1):
            top += w[7][:, :, dy + 1, dx + 1] @ imgc[:, dy, 1 + dx:255 + dx]
    for dy in (-1, 0):                     # d=1 (bottom row)
        for dx in (-1, 0, 1):
            bot += w[1][:, :, dy + 1, dx + 1] @ imgc[:, 255 + dy,
                                                     1 + dx:255 + dx]
    for dy in (-1, 0, 1):                  # d=5 (left col)
        for dx in (0, 1):
            lef += w[5][:, :, dy + 1, dx + 1] @ imgc[:, 1 + dy:255 + dy, dx]
    for dy in (-1, 0, 1):                  # d=3 (right col)
        for dx in (-1, 0):
            rig += w[3][:, :, dy + 1, dx + 1] @ imgc[:, 1 + dy:255 + dy,
                                                     255 + dx]
    top += bias[7][:, None]
    bot += bias[1][:, None]
    lef += bias[5][:, None]
    rig += bias[3][:, None]
    cspec = [(8, (1, 3), (1, 3), (0, 2), (0, 2)),
             (6, (1, 3), (0, 2), (0, 2), (254, 256)),
             (2, (0, 2), (1, 3), (254, 256), (0, 2)),
             (0, (0, 2), (0, 2), (254, 256), (254, 256))]
    corners = np.zeros((4, 64), np.float32)
    for i, (d, ky, kx, iy, ix) in enumerate(cspec):
        corners[i] = np.einsum(
            'oikl,ikl->o', w[d][:, :, ky[0]:ky[1], kx[0]:kx[1]],
            imgc[:, iy[0]:iy[1], ix[0]:ix[1]]) + bias[d]
    return top, bot, lef, rig, corners


def _make_in_maps(img, weights, bias):
    img = np.asarray(img, np.float32)
    wt = _prep_wt(np.asarray(weights, np.float32))
    bs = _prep_bias(np.asarray(bias, np.float32))
    return [{"img_prep": _prep_img(img[c]), "wt": wt, "bias": bs}
            for c in range(NCORES)]


def _unprep_out(o, brd):
    """Assemble [C,H,W] from dense out + host border overlay."""
    v = o.astype(np.float32).reshape(NSTRIP, 2, 2, 64, NG, 2, 256)
    out = np.ascontiguousarray(
        v.transpose(3, 0, 1, 4, 2, 5, 6).reshape(C, H, W))
    top, bot, lef, rig, corners = brd
    out[:, 0, 1:255] = top
    out[:, 255, 1:255] = bot
    out[:, 1:255, 0] = lef
    out[:, 1:255, 255] = rig
    out[:, 0, 0] = corners[0]
    out[:, 0, 255] = corners[1]
    out[:, 255, 0] = corners[2]
    out[:, 255, 255] = corners[3]
    return out


def _assemble(res, img, weights, bias):
    img = np.asarray(img, np.float32)
    weights = np.asarray(weights, np.float32)
    bias = np.asarray(bias, np.float32)
    return np.stack([
        _unprep_out(res.results[c]["out"], _borders(img[c], weights, bias))
        for c in range(NCORES)])


def kernel(img, weights, bias):
    nc = _get_nc()
    in_maps = _make_in_maps(img, weights, bias)
    res = run_bass_kernel_spmd(nc, in_maps, list(range(NCORES)))
    return _assemble(res, img, weights, bias)
